# revision 1
# baseline (speedup 1.0000x reference)
"""GCN-GRU node-classification kernel for 8 TRN2 NeuronCores.

Node-sharded graph parallelism per the sharding hint:
- 6250 nodes/core (padded to 6272 = 49 blocks of 128); edges row-partitioned,
  row-sorted, per-block column-band split (band A: idx < pivot, band B:
  idx - pivot) so gather indices fit dma_gather's int16, padded to a uniform
  tile count per (block, band) so all 8 cores share one SPMD program.
- Per step: spmm1 gathers W1 rows (bf16 replicated) via dma_gather; scatter is
  PE one-hot matmuls (one-hot = iota==lrow built on DVE, edge val folded in);
  x1->y=x1@W2 fused per block; AllGather y; spmm2 gathers y (bf16, padded to
  128 cols); GRU pointwise per node in transposed [feat, node] layout.
- BatchNorm via AllReduce of per-core sums; attention readout via row/col
  gathers of final_emb + PE one-hot scatter; final MLP + log_softmax.
"""
import math
from contextlib import ExitStack
import numpy as np
import ml_dtypes

import concourse.bass as bass
import concourse.bacc as bacc
import concourse.mybir as mybir
import concourse.tile as tile
from concourse.bass_utils import run_bass_kernel_spmd

f32 = mybir.dt.float32
bf16 = mybir.dt.bfloat16
i16 = mybir.dt.int16
AF = mybir.ActivationFunctionType
OP = mybir.AluOpType
BF = ml_dtypes.bfloat16

P = 128
BN_EPS = 1e-5


# ----------------------------------------------------------------------------
# host-side preprocessing
# ----------------------------------------------------------------------------

def _wrap_idx(a):
    # idx stream -> [128, L/16] int16, wrapped in 16 partitions, replicated x8
    L = a.shape[0]
    w = a.reshape(L // 16, 16).T.astype(np.int16)  # [16, L/16]
    return np.tile(w, (8, 1)).copy()


def _wrap_val(a, dtype=np.float32):
    return np.ascontiguousarray(a.reshape(-1, P).T.astype(dtype))


class Meta:
    pass


def preprocess(inputs, n_cores=8):
    adj_idx = np.asarray(inputs["adj_idx"])
    adj_val = np.asarray(inputs["adj_val"])
    start_day = int(inputs["start_day"])
    end_day = int(inputs["end_day"])
    N = int(inputs["W1"].shape[0])
    T = end_day - start_day + 1

    m = Meta()
    m.N = N
    m.T = T
    m.NC = n_cores
    m.NL = N // n_cores                       # nodes per core
    assert m.NL * n_cores == N
    m.NB = math.ceil(m.NL / P)                # 128-blocks per core
    m.NBP = m.NB * P                          # padded nodes per core
    m.PIV1 = 32500 if N > 32768 else max(P, (N // 2) // P * P)

    def remap(c):
        return (c // m.NL) * m.NBP + (c % m.NL)

    m.PIV2 = int(remap(m.PIV1)) if m.PIV1 < N else n_cores * m.NBP
    assert m.PIV1 <= 32768 and (N - m.PIV1) <= 32767
    assert m.PIV2 <= 32768 and (n_cores * m.NBP - m.PIV2) <= 32767

    steps = [start_day + t for t in range(T)]
    att_day = end_day + 1

    TA = TB = TA7 = TB7 = 1
    percore_raw = []
    for k in range(n_cores):
        base = k * m.NL
        days = []
        for t in steps + [att_day]:
            row = adj_idx[t, 0]
            col = adj_idx[t, 1]
            sel = (row >= base) & (row < base + m.NL)
            if t == att_day:
                sel &= row != col
            r = (row[sel] - base).astype(np.int64)
            c = col[sel].astype(np.int64)
            if t == att_day:
                deg = np.bincount(r, minlength=m.NL).astype(np.float32)
                inv_deg = np.where(deg != 0, 1.0 / np.maximum(deg, 1.0), 1.0)
                v = inv_deg[r].astype(np.float32)
            else:
                v = adj_val[t][sel].astype(np.float32)
            o = np.argsort(r, kind="stable")
            r, c, v = r[o], c[o], v[o]
            blk = r >> 7
            A = c < m.PIV1
            na = np.bincount(blk[A], minlength=m.NB)
            nb = np.bincount(blk[~A], minlength=m.NB)
            ta = int(np.max((na + 127) // 128))
            tb = int(np.max((nb + 127) // 128))
            if t == att_day:
                TA7, TB7 = max(TA7, ta), max(TB7, tb)
            else:
                TA, TB = max(TA, ta), max(TB, tb)
            days.append((r, c, v))
        percore_raw.append(days)
    m.TA, m.TB, m.TA7, m.TB7 = TA, TB, TA7, TB7

    def build_day(r, c, v, ta, tb, with_row):
        LA, LB = m.NB * ta * P, m.NB * tb * P
        ia1 = np.zeros(LA, np.int64); ib1 = np.zeros(LB, np.int64)
        ia2 = np.zeros(LA, np.int64); ib2 = np.zeros(LB, np.int64)
        va = np.zeros(LA, np.float32); vb = np.zeros(LB, np.float32)
        la = np.zeros(LA, np.float32); lb = np.zeros(LB, np.float32)
        ra = np.zeros(LA, np.int64); rb = np.zeros(LB, np.int64)
        blk = r >> 7
        A = c < m.PIV1
        rm = (c // m.NL) * m.NBP + (c % m.NL)
        for b in range(m.NB):
            sb = blk == b
            for ii1, ii2, vv, ll, rr, tt, piv1, piv2, band in (
                    (ia1, ia2, va, la, ra, ta, 0, 0, A),
                    (ib1, ib2, vb, lb, rb, tb, m.PIV1, m.PIV2, ~A)):
                s = sb & band
                n = int(s.sum())
                bs = b * tt * P
                ii1[bs:bs + n] = c[s] - piv1
                ii2[bs:bs + n] = rm[s] - piv2
                vv[bs:bs + n] = v[s]
                ll[bs:bs + n] = (r[s] - (b << 7)).astype(np.float32)
                rr[bs:bs + n] = r[s]
        out = dict(
            ia1=_wrap_idx(ia1), ib1=_wrap_idx(ib1),
            ia2=_wrap_idx(ia2), ib2=_wrap_idx(ib2),
            va=_wrap_val(va, BF), vb=_wrap_val(vb, BF),
            vaf=_wrap_val(va), vbf=_wrap_val(vb),
            la=_wrap_val(la), lb=_wrap_val(lb),
        )
        if with_row:
            out["ra"] = _wrap_idx(ra)
            out["rb"] = _wrap_idx(rb)
        return out

    percore = []
    for k in range(n_cores):
        days = percore_raw[k]
        built = [build_day(*days[t], TA, TB, False) for t in range(T)]
        built.append(build_day(*days[T], TA7, TB7, True))
        percore.append(built)
    return m, percore


# ----------------------------------------------------------------------------
# device program
# ----------------------------------------------------------------------------

def build_program(m, NHID, NOUT, attn_b):
    NG = NOUT
    NB, TA, TB, TA7, TB7 = m.NB, m.TA, m.TB, m.TA7, m.TB7
    NBP, T, NC, N, NL = m.NBP, m.T, m.NC, m.N, m.NL

    CH = 7 if NB % 7 == 0 else 1
    NCHUNK = NB // CH

    nc = bacc.Bacc("TRN2", target_bir_lowering=False, debug=False,
                   num_devices=NC)

    def din(name, shape, dtype):
        return nc.dram_tensor(name, list(shape), dtype, kind="ExternalInput")

    W1bf = din("W1bf", [N, NHID], bf16)
    iota_in = din("iota", [P, P], f32)
    ident_in = din("ident", [P, P], f32)
    W2_in = din("W2bf", [NHID, NOUT], bf16)
    wihrz_in = din("wihrz", [NOUT, 2 * NG], bf16)
    whhrz_in = din("whhrz", [NG, 2 * NG], bf16)
    wihn_in = din("wihn", [NOUT, NG], bf16)
    whhn_in = din("whhn", [NG, NG], bf16)
    npw1_in = din("npw1", [2 * NG, NG], bf16)
    npw2_in = din("npw2", [NG, 2], bf16)
    b1_in = din("b1", [NHID, 1], f32)
    b2_in = din("b2", [NOUT, 1], f32)
    brz_in = din("brz", [2 * NG, 1], f32)
    brzz_in = din("brzz", [NG, 1], f32)
    bihn_in = din("bihn", [NG, 1], f32)
    bhhn_in = din("bhhn", [NG, 1], f32)
    npb1_in = din("npb1", [NG, 1], f32)
    npb2a_in = din("npb2a", [1, 1], f32)
    npb2b_in = din("npb2b", [1, 1], f32)
    bng_in = din("bng", [NG, 1], f32)
    bnb_in = din("bnb", [NG, 1], f32)
    a1_in = din("a1rep", [P, NG], f32)
    a2_in = din("a2rep", [P, NG], f32)

    LA, LB = NB * TA * P, NB * TB * P
    LA7, LB7 = NB * TA7 * P, NB * TB7 * P
    ia1_d = din("ia1", [T, P, LA // 16], i16)
    ib1_d = din("ib1", [T, P, LB // 16], i16)
    ia2_d = din("ia2", [T, P, LA // 16], i16)
    ib2_d = din("ib2", [T, P, LB // 16], i16)
    va_d = din("va", [T, P, LA // P], bf16)
    vb_d = din("vb", [T, P, LB // P], bf16)
    la_d = din("la", [T, P, LA // P], f32)
    lb_d = din("lb", [T, P, LB // P], f32)
    i7a_d = din("i7a", [P, LA7 // 16], i16)
    i7b_d = din("i7b", [P, LB7 // 16], i16)
    i7ra_d = din("i7ra", [P, LA7 // 16], i16)
    i7rb_d = din("i7rb", [P, LB7 // 16], i16)
    v7a_d = din("v7a", [P, LA7 // P], f32)
    v7b_d = din("v7b", [P, LB7 // P], f32)
    l7a_d = din("l7a", [P, LA7 // P], f32)
    l7b_d = din("l7b", [P, LB7 // P], f32)

    pred_out = nc.dram_tensor("pred", [2, NL], f32, kind="ExternalOutput")

    rg = [list(range(NC))]

    with tile.TileContext(nc) as tc, ExitStack() as es:
        pp = es.enter_context(tc.tile_pool(name="persist", bufs=1))
        dram = es.enter_context(tc.tile_pool(name="dram", bufs=1, space="DRAM"))
        sp = es.enter_context(tc.tile_pool(name="work", bufs=2))
        scr = es.enter_context(tc.tile_pool(name="scr", bufs=1))

        def ld(src, shape, dtype):
            t_ = pp.tile(shape, dtype, name=src.name, tag=src.name)
            nc.sync.dma_start(t_[:], src[:])
            return t_

        iota = ld(iota_in, [P, P], f32)
        ident = ld(ident_in, [P, P], f32)
        W2 = ld(W2_in, [NHID, NOUT], bf16)
        wihrz = ld(wihrz_in, [NOUT, 2 * NG], bf16)
        whhrz = ld(whhrz_in, [NG, 2 * NG], bf16)
        wihn = ld(wihn_in, [NOUT, NG], bf16)
        whhn = ld(whhn_in, [NG, NG], bf16)
        npw1 = ld(npw1_in, [2 * NG, NG], bf16)
        npw2 = ld(npw2_in, [NG, 2], bf16)
        b1 = ld(b1_in, [NHID, 1], f32)
        b2 = ld(b2_in, [NOUT, 1], f32)
        brz = ld(brz_in, [2 * NG, 1], f32)
        brzz = ld(brzz_in, [NG, 1], f32)
        bihn = ld(bihn_in, [NG, 1], f32)
        bhhn = ld(bhhn_in, [NG, 1], f32)
        npb1 = ld(npb1_in, [NG, 1], f32)
        npb2a = ld(npb2a_in, [1, 1], f32)
        npb2b = ld(npb2b_in, [1, 1], f32)
        bng = ld(bng_in, [NG, 1], f32)
        bnb = ld(bnb_in, [NG, 1], f32)
        a1rep = ld(a1_in, [P, NG], f32)
        a2rep = ld(a2_in, [P, NG], f32)

        epsap = pp.tile([NG, 1], f32)
        nc.vector.memset(epsap[:], BN_EPS)
        attnbap = pp.tile([P, 1], f32)
        nc.vector.memset(attnbap[:], attn_b)
        h = pp.tile([NG, NBP], f32)
        nc.vector.memset(h[:], 0.0)
        x2bf = pp.tile([NOUT, NBP], bf16)
        zT = pp.tile([2 * NG, NBP], bf16)
        ystage = pp.tile([P, NB, NHID], bf16)
        nc.vector.memset(ystage[:], 0.0)

        y_in = [dram.tile([NBP, NHID], bf16, name=f"y_in{i}") for i in range(T)]
        y_full = [dram.tile([NC * NBP, NHID], bf16, addr_space="Shared",
                            name=f"y_full{i}") for i in range(T)]
        femb_loc = dram.tile([NBP, NHID], bf16)
        femb_full = dram.tile([NC * NBP, NHID], bf16, addr_space="Shared")
        bn_in = dram.tile([NG, 2], f32)
        bn_out = dram.tile([NG, 2], f32, addr_space="Shared")

        vaS = pp.tile([P, LA // P], bf16)
        vbS = pp.tile([P, LB // P], bf16)
        laS = pp.tile([P, LA // P], f32)
        lbS = pp.tile([P, LB // P], f32)

        def onehot(dst, lr_sl, val_sl):
            nt = dst.shape[1]
            nc.vector.tensor_tensor(
                out=dst[:], in0=iota[:, None, :].to_broadcast([P, nt, P]),
                in1=lr_sl[:, :, None].to_broadcast([P, nt, P]),
                op=OP.is_equal)
            if val_sl is not None:
                nc.vector.tensor_tensor(
                    out=dst[:], in0=dst[:],
                    in1=val_sl[:, :, None].to_broadcast([P, nt, P]),
                    op=OP.mult)

        def gather(dst, src_ap, idx_dram, off16, n16, nidx, elem, tag):
            # single_packet coalesces each engine's descs into one packet
            # (<=64 descs) -> cap each call at 1024 indices
            ix = sp.tile([P, n16], i16, tag=tag)
            nc.sync.dma_start(ix[:], idx_dram[:, off16:off16 + n16])
            nt = nidx // P
            SUB = 8
            for s0 in range(0, nt, SUB):
                st = min(SUB, nt - s0)
                nc.gpsimd.dma_gather(dst[:, s0:s0 + st, :], src_ap,
                                     ix[:, s0 * 8:(s0 + st) * 8],
                                     st * P, st * P, elem)

        def spmm(t, ps, ia_d, ib_d, srcA, srcB, elem, out_cb, tag_pb, pdim):
            """Band-split gather + one-hot matmul scatter over all blocks."""
            for ch in range(NCHUNK):
                ntA, ntB = CH * TA, CH * TB
                gA = sp.tile([P, ntA, elem], bf16, tag="gA")
                gather(gA, srcA, ia_d[t], ch * ntA * 8, ntA * 8,
                       ntA * P, elem, "ixA")
                gB = sp.tile([P, ntB, elem], bf16, tag="gB")
                gather(gB, srcB, ib_d[t], ch * ntB * 8, ntB * 8,
                       ntB * P, elem, "ixB")
                ohA = sp.tile([P, ntA, P], bf16, tag="ohA")
                onehot(ohA, laS[:, ch * ntA:(ch + 1) * ntA],
                       vaS[:, ch * ntA:(ch + 1) * ntA])
                ohB = sp.tile([P, ntB, P], bf16, tag="ohB")
                onehot(ohB, lbS[:, ch * ntB:(ch + 1) * ntB],
                       vbS[:, ch * ntB:(ch + 1) * ntB])
                for j in range(CH):
                    b = ch * CH + j
                    pb = ps.tile([pdim, P], f32, tag=tag_pb, space="PSUM")
                    for a in range(TA):
                        nc.tensor.matmul(
                            pb[:], lhsT=gA[:, j * TA + a, :pdim],
                            rhs=ohA[:, j * TA + a, :],
                            start=(a == 0), stop=False)
                    for bb in range(TB):
                        nc.tensor.matmul(
                            pb[:], lhsT=gB[:, j * TB + bb, :pdim],
                            rhs=ohB[:, j * TB + bb, :],
                            start=False, stop=(bb == TB - 1))
                    out_cb(b, pb)

        # ================= time steps =================
        for t in range(T):
            nc.sync.dma_start(vaS[:], va_d[t])
            nc.sync.dma_start(vbS[:], vb_d[t])
            nc.sync.dma_start(laS[:], la_d[t])
            nc.sync.dma_start(lbS[:], lb_d[t])

            # ---- spmm1 + fused y = relu(.)@W2, transposed staging ----
            with tc.tile_pool(name=f"ps1_{t}", bufs=2, space="PSUM") as ps:
                def close1(b, pb, ps=ps):
                    x1b = sp.tile([NHID, P], bf16, tag="x1b")
                    nc.scalar.activation(x1b[:], pb[:], AF.Relu, bias=b1[:])
                    py = ps.tile([NOUT, P], f32, tag="py", space="PSUM")
                    nc.tensor.matmul(py[:], lhsT=W2[:], rhs=x1b[:],
                                     start=True, stop=True)
                    ysb = sp.tile([NOUT, P], f32, tag="ysb")
                    nc.scalar.copy(ysb[:], py[:])
                    pyt = ps.tile([P, NOUT], f32, tag="pyt", space="PSUM")
                    nc.tensor.transpose(pyt[:], ysb[:], ident[:NOUT, :NOUT])
                    nc.scalar.copy(ystage[:, b, :NOUT], pyt[:])
                spmm(t, ps, ia1_d, ib1_d, W1bf[:, :], W1bf[m.PIV1:, :],
                     NHID, close1, "pb", NHID)

            nc.sync.dma_start(
                y_in[t][:].rearrange("(b p) d -> p b d", p=P), ystage[:])
            nc.gpsimd.collective_compute(
                "AllGather", OP.bypass, replica_groups=rg,
                ins=[y_in[t].opt()], outs=[y_full[t].opt()])

            # ---- spmm2 ----
            with tc.tile_pool(name=f"ps2_{t}", bufs=2, space="PSUM") as ps:
                yf = y_full[t]
                def close2(b, pb):
                    nc.scalar.activation(
                        x2bf[:, b * P:(b + 1) * P], pb[:], AF.Identity,
                        bias=b2[:])
                spmm(t, ps, ia2_d, ib2_d, yf[:, :], yf[m.PIV2:, :],
                     NHID, close2, "pb2", NOUT)

            # ---- GRU ----
            with tc.tile_pool(name=f"psg_{t}", bufs=2, space="PSUM") as ps:
                CL = 512
                for s in range(0, NBP, CL):
                    L = min(CL, NBP - s)
                    hbfc = scr.tile([NG, CL], bf16, tag="hbfc")
                    nc.scalar.copy(hbfc[:, :L], h[:, s:s + L])
                    prz = ps.tile([2 * NG, CL], f32, tag="prz", space="PSUM")
                    nc.tensor.matmul(prz[:, :L], lhsT=wihrz[:],
                                     rhs=x2bf[:, s:s + L], start=True,
                                     stop=False)
                    nc.tensor.matmul(prz[:, :L], lhsT=whhrz[:],
                                     rhs=hbfc[:, :L], start=False,
                                     stop=True)
                    rzr = sp.tile([NG, CL], f32, tag="rzr")
                    nc.scalar.activation(rzr[:, :L], prz[:NG, :L], AF.Sigmoid,
                                         bias=brz[:NG])
                    rzz = sp.tile([NG, CL], f32, tag="rzz")
                    nc.scalar.activation(rzz[:, :L], prz[NG:, :L], AF.Sigmoid,
                                         bias=brzz[:])
                    pn = ps.tile([NG, CL], f32, tag="pn", space="PSUM")
                    nc.tensor.matmul(pn[:, :L], lhsT=wihn[:],
                                     rhs=x2bf[:, s:s + L], start=True,
                                     stop=True)
                    phn = ps.tile([NG, CL], f32, tag="phn", space="PSUM")
                    nc.tensor.matmul(phn[:, :L], lhsT=whhn[:],
                                     rhs=hbfc[:, :L], start=True,
                                     stop=True)
                    ghn = scr.tile([NG, CL], f32, tag="ghn")
                    nc.scalar.activation(ghn[:, :L], phn[:, :L], AF.Identity,
                                         bias=bhhn[:])
                    t1 = scr.tile([NG, CL], f32, tag="t1")
                    nc.vector.tensor_tensor(out=t1[:, :L], in0=rzr[:, :L],
                                            in1=ghn[:, :L], op=OP.mult)
                    t2 = scr.tile([NG, CL], f32, tag="t2")
                    nc.vector.tensor_tensor(out=t2[:, :L], in0=t1[:, :L],
                                            in1=pn[:, :L], op=OP.add)
                    nsb = scr.tile([NG, CL], f32, tag="nsb")
                    nc.scalar.activation(nsb[:, :L], t2[:, :L], AF.Tanh,
                                         bias=bihn[:])
                    dd = scr.tile([NG, CL], f32, tag="t2", name="dd")
                    nc.vector.tensor_tensor(out=dd[:, :L], in0=h[:, s:s + L],
                                            in1=nsb[:, :L], op=OP.subtract)
                    zd = scr.tile([NG, CL], f32, tag="t1", name="zd")
                    nc.vector.tensor_tensor(out=zd[:, :L], in0=rzz[:, :L],
                                            in1=dd[:, :L], op=OP.mult)
                    nc.vector.tensor_tensor(out=h[:, s:s + L], in0=nsb[:, :L],
                                            in1=zd[:, :L], op=OP.add)

        # ================= BatchNorm =================
        hsum = pp.tile([NG, 1], f32)
        nc.vector.tensor_reduce(out=hsum[:], in_=h[:, :NL],
                                axis=mybir.AxisListType.X, op=OP.add)
        hsq = pp.tile([NG, 1], f32)
        nc.scalar.activation(x2bf[:, :NL], h[:, :NL], AF.Square,
                             accum_out=hsq[:])
        bnsb = pp.tile([NG, 2], f32)
        nc.vector.tensor_copy(bnsb[:, 0:1], hsum[:])
        nc.vector.tensor_copy(bnsb[:, 1:2], hsq[:])
        nc.sync.dma_start(bn_in[:], bnsb[:])
        nc.gpsimd.collective_compute(
            "AllReduce", OP.add, replica_groups=rg,
            ins=[bn_in.opt()], outs=[bn_out.opt()])
        bnrs = pp.tile([NG, 2], f32)
        nc.sync.dma_start(bnrs[:], bn_out[:])
        mean = pp.tile([NG, 1], f32)
        nc.scalar.mul(mean[:], bnrs[:, 0:1], 1.0 / N)
        ex2 = pp.tile([NG, 1], f32)
        nc.scalar.mul(ex2[:], bnrs[:, 1:2], 1.0 / N)
        msq = pp.tile([NG, 1], f32)
        nc.scalar.activation(msq[:], mean[:], AF.Square)
        var = pp.tile([NG, 1], f32)
        nc.vector.tensor_tensor(out=var[:], in0=ex2[:], in1=msq[:],
                                op=OP.subtract)
        sd = pp.tile([NG, 1], f32)
        nc.scalar.activation(sd[:], var[:], AF.Sqrt, bias=epsap[:])
        inv = pp.tile([NG, 1], f32)
        nc.vector.reciprocal(inv[:], sd[:])
        scale = pp.tile([NG, 1], f32)
        nc.vector.tensor_tensor(out=scale[:], in0=bng[:], in1=inv[:],
                                op=OP.mult)
        mscale = pp.tile([NG, 1], f32)
        nc.vector.tensor_tensor(out=mscale[:], in0=mean[:], in1=scale[:],
                                op=OP.mult)
        shift = pp.tile([NG, 1], f32)
        nc.vector.tensor_tensor(out=shift[:], in0=bnb[:], in1=mscale[:],
                                op=OP.subtract)
        nc.scalar.activation(h[:], h[:], AF.Identity, bias=shift[:],
                             scale=scale[:])
        nc.scalar.copy(zT[:NG, :], h[:])
        with tc.tile_pool(name="psT", bufs=2, space="PSUM") as psT:
            for b in range(NB):
                pyt = psT.tile([P, NG], f32, tag="pyt2", space="PSUM")
                nc.tensor.transpose(pyt[:], h[:, b * P:(b + 1) * P],
                                    ident[:NG, :NG])
                nc.scalar.copy(ystage[:, b, :NOUT], pyt[:])
        nc.sync.dma_start(
            femb_loc[:].rearrange("(b p) d -> p b d", p=P), ystage[:])
        nc.gpsimd.collective_compute(
            "AllGather", OP.bypass, replica_groups=rg,
            ins=[femb_loc.opt()], outs=[femb_full.opt()])

        # ================= attention readout =================
        v7aS = pp.tile([P, LA7 // P], f32)
        v7bS = pp.tile([P, LB7 // P], f32)
        l7aS = pp.tile([P, LA7 // P], f32)
        l7bS = pp.tile([P, LB7 // P], f32)
        nc.sync.dma_start(v7aS[:], v7a_d[:])
        nc.sync.dma_start(v7bS[:], v7b_d[:])
        nc.sync.dma_start(l7aS[:], l7a_d[:])
        nc.sync.dma_start(l7bS[:], l7b_d[:])

        with tc.tile_pool(name="psA", bufs=2, space="PSUM") as ps:
            for ch in range(NCHUNK):
                tiles = {}
                for sfx, nt, tt, icol, irow, vS, lS, src in (
                        ("A", CH * TA7, TA7, i7a_d, i7ra_d, v7aS, l7aS,
                         femb_full[:, :]),
                        ("B", CH * TB7, TB7, i7b_d, i7rb_d, v7bS, l7bS,
                         femb_full[m.PIV2:, :])):
                    gC = sp.tile([P, nt, NHID], bf16, tag="g" + sfx)
                    gather(gC, src, icol, ch * nt * 8, nt * 8, nt * P, NHID,
                           "ix" + sfx)
                    gR = scr.tile([P, nt, NHID], bf16, tag="gR" + sfx)
                    gather(gR, femb_loc[:, :], irow, ch * nt * 8, nt * 8,
                           nt * P, NHID, "ixr" + sfx)
                    oh = sp.tile([P, nt, P], bf16, tag="oh" + sfx)
                    onehot(oh, lS[:, ch * nt:(ch + 1) * nt], None)
                    mm = scr.tile([P, nt, NOUT], bf16, tag="mscr")
                    nc.vector.tensor_tensor(
                        out=mm[:], in0=gR[:, :, :NOUT],
                        in1=a1rep[:, None, :].to_broadcast([P, nt, NOUT]),
                        op=OP.mult)
                    s1 = sp.tile([P, nt], f32, tag="s1")
                    nc.vector.tensor_reduce(out=s1[:], in_=mm[:],
                                            axis=mybir.AxisListType.X,
                                            op=OP.add)
                    nc.vector.tensor_tensor(
                        out=mm[:], in0=gC[:, :, :NOUT],
                        in1=a2rep[:, None, :].to_broadcast([P, nt, NOUT]),
                        op=OP.mult)
                    s2 = sp.tile([P, nt], f32, tag="s2")
                    nc.vector.tensor_reduce(out=s2[:], in_=mm[:],
                                            axis=mybir.AxisListType.X,
                                            op=OP.add)
                    nc.vector.tensor_tensor(out=s1[:], in0=s1[:], in1=s2[:],
                                            op=OP.add)
                    wv = sp.tile([P, nt], f32, tag="wv" + sfx)
                    nc.scalar.activation(wv[:], s1[:], AF.Sigmoid,
                                         bias=attnbap[:])
                    nc.vector.tensor_tensor(
                        out=wv[:], in0=wv[:],
                        in1=vS[:, ch * nt:(ch + 1) * nt], op=OP.mult)
                    for ti in range(nt):
                        nc.scalar.activation(gC[:, ti, NOUT:2 * NOUT],
                                             gC[:, ti, :NOUT],
                                             AF.Copy, scale=wv[:, ti:ti + 1])
                    tiles[sfx] = (gC, oh, tt)
                for j in range(CH):
                    b = ch * CH + j
                    pnb = ps.tile([NOUT, P], f32, tag="pnb", space="PSUM")
                    cbf, oh, tt = tiles["A"]
                    for a in range(tt):
                        nc.tensor.matmul(
                            pnb[:], lhsT=cbf[:, j * tt + a, NOUT:2 * NOUT],
                            rhs=oh[:, j * tt + a, :],
                            start=(a == 0), stop=False)
                    cbf, oh, tt = tiles["B"]
                    for bb in range(tt):
                        nc.tensor.matmul(
                            pnb[:], lhsT=cbf[:, j * tt + bb, NOUT:2 * NOUT],
                            rhs=oh[:, j * tt + bb, :],
                            start=False, stop=(bb == tt - 1))
                    nc.scalar.copy(zT[NG:, b * P:(b + 1) * P], pnb[:])

        # ================= final MLP + log_softmax =================
        with tc.tile_pool(name="psF", bufs=2, space="PSUM") as ps:
            CL = 512
            for s in range(0, NBP, CL):
                L = min(CL, NBP - s)
                ph1 = ps.tile([NG, CL], f32, tag="ph1", space="PSUM")
                nc.tensor.matmul(ph1[:, :L], lhsT=npw1[:], rhs=zT[:, s:s + L],
                                 start=True, stop=True)
                h1b = sp.tile([NG, CL], bf16, tag="h1b")
                nc.scalar.activation(h1b[:, :L], ph1[:, :L], AF.Relu,
                                     bias=npb1[:])
                ps2a = ps.tile([1, CL], f32, tag="ps2a", space="PSUM")
                nc.tensor.matmul(ps2a[:, :L], lhsT=npw2[:, 0:1],
                                 rhs=h1b[:, :L], start=True, stop=True)
                s0 = scr.tile([1, CL], f32, tag="lsm_s0")
                nc.scalar.activation(s0[:, :L], ps2a[:, :L],
                                     AF.Identity, bias=npb2a[:])
                ps2b = ps.tile([1, CL], f32, tag="ps2b", space="PSUM")
                nc.tensor.matmul(ps2b[:, :L], lhsT=npw2[:, 1:2],
                                 rhs=h1b[:, :L], start=True, stop=True)
                s1c = scr.tile([1, CL], f32, tag="lsm_s1")
                nc.scalar.activation(s1c[:, :L], ps2b[:, :L],
                                     AF.Identity, bias=npb2b[:])
                if s >= NL:
                    continue
                Lv = min(L, NL - s)
                mx = scr.tile([1, CL], f32, tag="lsm_mx")
                nc.vector.tensor_tensor(out=mx[:, :L], in0=s0[:, :L],
                                        in1=s1c[:, :L], op=OP.max)
                sh0 = scr.tile([1, CL], f32, tag="lsm_sh0")
                nc.vector.tensor_tensor(out=sh0[:, :L], in0=s0[:, :L],
                                        in1=mx[:, :L], op=OP.subtract)
                sh1 = scr.tile([1, CL], f32, tag="lsm_sh1")
                nc.vector.tensor_tensor(out=sh1[:, :L], in0=s1c[:, :L],
                                        in1=mx[:, :L], op=OP.subtract)
                e0 = scr.tile([1, CL], f32, tag="lsm_s0")
                nc.scalar.activation(e0[:, :L], sh0[:, :L], AF.Exp)
                e1 = scr.tile([1, CL], f32, tag="lsm_s1")
                nc.scalar.activation(e1[:, :L], sh1[:, :L], AF.Exp)
                se = scr.tile([1, CL], f32, tag="lsm_mx")
                nc.vector.tensor_tensor(out=se[:, :L], in0=e0[:, :L],
                                        in1=e1[:, :L], op=OP.add)
                lg = scr.tile([1, CL], f32, tag="lsm_s0")
                nc.scalar.activation(lg[:, :L], se[:, :L], AF.Ln)
                p0 = scr.tile([1, CL], f32, tag="lsm_s1")
                nc.vector.tensor_tensor(out=p0[:, :L], in0=sh0[:, :L],
                                        in1=lg[:, :L], op=OP.subtract)
                p1 = scr.tile([1, CL], f32, tag="lsm_mx")
                nc.vector.tensor_tensor(out=p1[:, :L], in0=sh1[:, :L],
                                        in1=lg[:, :L], op=OP.subtract)
                nc.sync.dma_start(pred_out[0:1, s:s + Lv], p0[:, :Lv])
                nc.sync.dma_start(pred_out[1:2, s:s + Lv], p1[:, :Lv])


    nc.compile()
    return nc


# ----------------------------------------------------------------------------
# entry point
# ----------------------------------------------------------------------------

def make_in_maps(inputs, m, percore):
    W1 = np.asarray(inputs["W1"], np.float32)
    W2 = np.asarray(inputs["W2"], np.float32)
    NG = W2.shape[1]
    w_ih = np.asarray(inputs["w_ih"], np.float32)
    w_hh = np.asarray(inputs["w_hh"], np.float32)
    b_ih = np.asarray(inputs["b_ih"], np.float32)
    b_hh = np.asarray(inputs["b_hh"], np.float32)
    attn_w = np.asarray(inputs["attn_w"], np.float32)

    shared = {
        "W1bf": W1.astype(BF),
        "iota": np.broadcast_to(np.arange(P, dtype=np.float32), (P, P)).copy(),
        "ident": np.eye(P, dtype=np.float32),
        "W2bf": W2.astype(BF),
        "wihrz": np.ascontiguousarray(w_ih[:2 * NG].T).astype(BF),
        "whhrz": np.ascontiguousarray(w_hh[:2 * NG].T).astype(BF),
        "wihn": np.ascontiguousarray(w_ih[2 * NG:].T).astype(BF),
        "whhn": np.ascontiguousarray(w_hh[2 * NG:].T).astype(BF),
        "npw1": np.asarray(inputs["np_w1"], np.float32).astype(BF),
        "npw2": np.asarray(inputs["np_w2"], np.float32).astype(BF),
        "b1": np.asarray(inputs["b1"], np.float32).reshape(-1, 1),
        "b2": np.asarray(inputs["b2"], np.float32).reshape(-1, 1),
        "brz": (b_ih[:2 * NG] + b_hh[:2 * NG]).reshape(-1, 1),
        "brzz": (b_ih[NG:2 * NG] + b_hh[NG:2 * NG]).reshape(-1, 1),
        "bihn": b_ih[2 * NG:].reshape(-1, 1),
        "bhhn": b_hh[2 * NG:].reshape(-1, 1),
        "npb1": np.asarray(inputs["np_b1"], np.float32).reshape(-1, 1),
        "npb2a": np.asarray(inputs["np_b2"], np.float32).reshape(-1, 1)[0:1],
        "npb2b": np.asarray(inputs["np_b2"], np.float32).reshape(-1, 1)[1:2],
        "bng": np.asarray(inputs["bn_gamma"], np.float32).reshape(-1, 1),
        "bnb": np.asarray(inputs["bn_beta"], np.float32).reshape(-1, 1),
        "a1rep": np.broadcast_to(attn_w[:NG, 0], (P, NG)).copy(),
        "a2rep": np.broadcast_to(attn_w[NG:, 0], (P, NG)).copy(),
    }

    in_maps = []
    for k in range(m.NC):
        d = dict(shared)
        days = percore[k]
        for key in ("ia1", "ib1", "ia2", "ib2", "va", "vb", "la", "lb"):
            d[key] = np.stack([days[t][key] for t in range(m.T)])
        d7 = days[m.T]
        d["i7a"], d["i7b"] = d7["ia2"], d7["ib2"]
        d["i7ra"], d["i7rb"] = d7["ra"], d7["rb"]
        d["v7a"], d["v7b"] = d7["vaf"], d7["vbf"]
        d["l7a"], d["l7b"] = d7["la"], d7["lb"]
        in_maps.append(d)
    return in_maps


_CACHE = {}
LAST_RESULTS = None


def kernel(**inputs):
    n_cores = 8
    m, percore = preprocess(inputs, n_cores)
    in_maps = make_in_maps(inputs, m, percore)
    key = (m.N, m.T, m.TA, m.TB, m.TA7, m.TB7)
    if key not in _CACHE:
        NHID = int(np.asarray(inputs["W1"]).shape[1])
        NOUT = int(np.asarray(inputs["W2"]).shape[1])
        attn_b = float(np.asarray(inputs["attn_b"]).reshape(-1)[0])
        _CACHE[key] = build_program(m, NHID, NOUT, attn_b)
    nc = _CACHE[key]
    res = run_bass_kernel_spmd(nc, in_maps, list(range(n_cores)))
    global LAST_RESULTS
    LAST_RESULTS = res
    pred = np.concatenate(
        [res.results[k]["pred"].T for k in range(n_cores)], axis=0)
    return np.ascontiguousarray(pred.astype(np.float32))


if __name__ == "__main__":
    import reference as R
    inputs = {k: np.asarray(v) for k, v in R.setup_inputs().items()}
    out = kernel(**inputs)
    print(out.shape, out.dtype, out[:2])



# revision 8
# speedup vs baseline: 3.5577x; 3.5577x over previous
"""GCN-GRU node-classification kernel for 8 TRN2 NeuronCores.

Node-sharded graph parallelism (6250 nodes/core, padded to 6272 = 49 blocks
of 128). Edges row-partitioned, row-sorted, per-block column-band split
(band A: remapped col < PIV2, band B: col - PIV2) so gather indices fit
dma_gather's int16, padded to a uniform tile count per (block, band) so all
8 cores share one SPMD program.

Host->device transfer is the wall-clock bottleneck on this setup (~55 MB/s
effective, ~50-100 ms per array), so inputs are packed into just three
arrays per core:
  - blobi [16, WI] int16: all gather indices, compact (the 16->128 partition
    replication dma_gather requires is done on device with 8 small DMAs).
  - blobb [128, WBF] bf16: per-day edge vals + one-hot row labels, attention
    day vals/labels, and every weight/bias/constant (converted to f32 on
    device where needed; labels/iota/ident are integer-exact in bf16).
  - w1s [6272, 128] bf16: this core's W1 row shard. An on-device AllGather
    materializes W1 in the *remapped* node layout [8*6272, 128], so spmm1
    gathers W1 with the same remapped indices spmm2 uses for y -> only one
    index set per day is transferred.

Per step: spmm1 gathers W1 rows via dma_gather; scatter is PE one-hot
matmuls (one-hot = iota==label built on DVE, edge val folded in);
x1->y=relu(x1)@W2 fused per block; AllGather y; spmm2 gathers y; GRU
pointwise per node in transposed [feat, node] layout. BatchNorm via
AllReduce of per-core sums; attention readout via row/col gathers of
final_emb + PE one-hot scatter; final MLP + log_softmax.
"""
import math
from contextlib import ExitStack
import numpy as np
import ml_dtypes

import concourse.bass as bass
import concourse.bacc as bacc
import concourse.mybir as mybir
import concourse.tile as tile
from concourse.bass_utils import run_bass_kernel_spmd

f32 = mybir.dt.float32
bf16 = mybir.dt.bfloat16
i16 = mybir.dt.int16
AF = mybir.ActivationFunctionType
OP = mybir.AluOpType
BF = ml_dtypes.bfloat16

P = 128
BN_EPS = 1e-5


class Meta:
    pass


# ----------------------------------------------------------------------------
# blob layouts (shared by host packing and device program)
# ----------------------------------------------------------------------------

def idx_layout(m):
    """Column offsets into blobi [16, WI] (int16)."""
    WA, WB = m.NB * m.TA * 8, m.NB * m.TB * 8
    WA7, WB7 = m.NB * m.TA7 * 8, m.NB * m.TB7 * 8
    off = {}
    c = 0
    for t in range(m.T):
        off[("ia", t)] = c; c += WA
        off[("ib", t)] = c; c += WB
    off["i7a"] = c; c += WA7
    off["i7b"] = c; c += WB7
    off["i7ra"] = c; c += WA7
    off["i7rb"] = c; c += WB7
    return off, c


def val_layout(m):
    """Column offsets into blobb [128, WBF] (bf16)."""
    CA, CB = m.NB * m.TA, m.NB * m.TB
    CA7, CB7 = m.NB * m.TA7, m.NB * m.TB7
    off = {}
    c = 0
    for t in range(m.T):
        off[("va", t)] = c; c += CA
        off[("vb", t)] = c; c += CB
        off[("la", t)] = c; c += CA
        off[("lb", t)] = c; c += CB
    for k, w in (("v7a", CA7), ("v7b", CB7), ("l7a", CA7), ("l7b", CB7),
                 ("W2", 64), ("wihrz", 128), ("whhrz", 128), ("wihn", 64),
                 ("whhn", 64), ("npw1", 64), ("npw2", 2), ("iota", 128),
                 ("ident", 128), ("a1rep", 64), ("a2rep", 64), ("b1", 1),
                 ("brz", 1), ("b2", 1), ("brzz", 1), ("bihn", 1),
                 ("bhhn", 1), ("npb1", 1), ("bng", 1), ("bnb", 1),
                 ("npb2", 1)):
        off[k] = c; c += w
    return off, c


# ----------------------------------------------------------------------------
# host-side preprocessing
# ----------------------------------------------------------------------------

def preprocess(inputs, n_cores=8):
    adj_idx = np.asarray(inputs["adj_idx"])
    adj_val = np.asarray(inputs["adj_val"])
    start_day = int(inputs["start_day"])
    end_day = int(inputs["end_day"])
    N = int(inputs["W1"].shape[0])
    T = end_day - start_day + 1

    m = Meta()
    m.N = N
    m.T = T
    m.NC = n_cores
    m.NL = N // n_cores                       # nodes per core
    assert m.NL * n_cores == N
    m.NB = math.ceil(m.NL / P)                # 128-blocks per core
    m.NBP = m.NB * P                          # padded nodes per core
    m.PIV1 = 32500 if N > 32768 else max(P, (N // 2) // P * P)

    def remap(c):
        return (c // m.NL) * m.NBP + (c % m.NL)

    m.PIV2 = int(remap(m.PIV1)) if m.PIV1 < N else n_cores * m.NBP
    assert m.PIV2 <= 32767 and (n_cores * m.NBP - m.PIV2) <= 32767

    steps = [start_day + t for t in range(T)]
    att_day = end_day + 1

    # pass 1: select per (core, day), compute band/block + tile counts
    TA = TB = TA7 = TB7 = 1
    percore_raw = []
    for k in range(n_cores):
        base = k * m.NL
        days = []
        for t in steps + [att_day]:
            row = adj_idx[t, 0]
            col = adj_idx[t, 1]
            sel = (row >= base) & (row < base + m.NL)
            if t == att_day:
                sel &= row != col
            r = (row[sel] - base).astype(np.int64)
            c = col[sel].astype(np.int64)
            if t == att_day:
                deg = np.bincount(r, minlength=m.NL).astype(np.float32)
                inv_deg = np.where(deg != 0, 1.0 / np.maximum(deg, 1.0), 1.0)
                v = inv_deg[r].astype(np.float32)
            else:
                v = adj_val[t][sel].astype(np.float32)
            o = np.argsort(r, kind="stable")
            r, c, v = r[o], c[o], v[o]
            blk = r >> 7
            A = c < m.PIV1
            na = np.bincount(blk[A], minlength=m.NB)
            nb = np.bincount(blk[~A], minlength=m.NB)
            ta = int(np.max((na + 127) // 128))
            tb = int(np.max((nb + 127) // 128))
            if t == att_day:
                TA7, TB7 = max(TA7, ta), max(TB7, tb)
            else:
                TA, TB = max(TA, ta), max(TB, tb)
            days.append((r, c, v))
        percore_raw.append(days)
    m.TA, m.TB, m.TA7, m.TB7 = TA, TB, TA7, TB7

    ioff, m.WI = idx_layout(m)
    voff, m.WBF = val_layout(m)

    def fill_band(r, c_rm, v, mask, tt, piv2):
        """Scatter band edges into padded slot streams (idx, val, label)."""
        L = m.NB * tt * P
        ii = np.zeros(L, np.int16)
        vv = np.zeros(L, np.float32)
        ll = np.zeros(L, np.float32)
        eb = blkv = None
        rb, cb, vb = r[mask], c_rm[mask], v[mask]
        eb = rb >> 7                      # sorted (r sorted)
        cnt = np.bincount(eb, minlength=m.NB)
        cum = np.concatenate(([0], np.cumsum(cnt)[:-1]))
        pos = eb * (tt * P) + (np.arange(len(eb)) - cum[eb])
        ii[pos] = (cb - piv2).astype(np.int16)
        vv[pos] = vb
        ll[pos] = (rb & 127).astype(np.float32)
        return ii, vv, ll, pos, rb

    def wrap_idx(a):
        return a.reshape(-1, 16).T         # [16, L/16]

    def wrap_val(a):
        return a.reshape(-1, P).T          # [128, L/128]

    percore = []
    for k in range(n_cores):
        blobi = np.zeros((16, m.WI), np.int16)
        blobb = np.zeros((P, m.WBF), BF)
        days = percore_raw[k]
        for t in range(T):
            r, c, v = days[t]
            rm = (c // m.NL) * m.NBP + (c % m.NL)
            A = rm < m.PIV2
            WA, CA = m.NB * TA * 8, m.NB * TA
            WB, CB = m.NB * TB * 8, m.NB * TB
            ii, vv, ll, _, _ = fill_band(r, rm, v, A, TA, 0)
            blobi[:, ioff[("ia", t)]:ioff[("ia", t)] + WA] = wrap_idx(ii)
            blobb[:, voff[("va", t)]:voff[("va", t)] + CA] = wrap_val(vv)
            blobb[:, voff[("la", t)]:voff[("la", t)] + CA] = wrap_val(ll)
            ii, vv, ll, _, _ = fill_band(r, rm, v, ~A, TB, m.PIV2)
            blobi[:, ioff[("ib", t)]:ioff[("ib", t)] + WB] = wrap_idx(ii)
            blobb[:, voff[("vb", t)]:voff[("vb", t)] + CB] = wrap_val(vv)
            blobb[:, voff[("lb", t)]:voff[("lb", t)] + CB] = wrap_val(ll)
        # attention day
        r, c, v = days[T]
        rm = (c // m.NL) * m.NBP + (c % m.NL)
        A = rm < m.PIV2
        WA7, CA7 = m.NB * TA7 * 8, m.NB * TA7
        WB7, CB7 = m.NB * TB7 * 8, m.NB * TB7
        for mask, tt, piv2, ki, kv, kl, kr, W_, C_ in (
                (A, TA7, 0, "i7a", "v7a", "l7a", "i7ra", WA7, CA7),
                (~A, TB7, m.PIV2, "i7b", "v7b", "l7b", "i7rb", WB7, CB7)):
            ii, vv, ll, pos, rb = fill_band(r, rm, v, mask, tt, piv2)
            rr = np.zeros(m.NB * tt * P, np.int16)
            rr[pos] = rb.astype(np.int16)
            blobi[:, ioff[ki]:ioff[ki] + W_] = wrap_idx(ii)
            blobi[:, ioff[kr]:ioff[kr] + W_] = wrap_idx(rr)
            blobb[:, voff[kv]:voff[kv] + C_] = wrap_val(vv)
            blobb[:, voff[kl]:voff[kl] + C_] = wrap_val(ll)
        percore.append((blobi, blobb))
    return m, percore


# ----------------------------------------------------------------------------
# device program
# ----------------------------------------------------------------------------

def build_program(m, NHID, NOUT, attn_b):
    NG = NOUT
    NB, TA, TB, TA7, TB7 = m.NB, m.TA, m.TB, m.TA7, m.TB7
    NBP, T, NC, N, NL = m.NBP, m.T, m.NC, m.N, m.NL
    ioff, WI = idx_layout(m)
    voff, WBF = val_layout(m)

    CH = 7 if NB % 7 == 0 else 1
    NCHUNK = NB // CH

    nc = bacc.Bacc("TRN2", target_bir_lowering=False, debug=False,
                   num_devices=NC)

    blobi = nc.dram_tensor("blobi", [16, WI], i16, kind="ExternalInput")
    blobb = nc.dram_tensor("blobb", [P, WBF], bf16, kind="ExternalInput")
    w1s = nc.dram_tensor("w1s", [NBP, NHID], bf16, kind="ExternalInput")
    pred_out = nc.dram_tensor("pred", [2, NL], f32, kind="ExternalOutput")

    rg = [list(range(NC))]

    CA, CB = NB * TA, NB * TB
    CA7, CB7 = NB * TA7, NB * TB7
    WA, WB = NB * TA * 8, NB * TB * 8
    WA7, WB7 = NB * TA7 * 8, NB * TB7 * 8

    with tile.TileContext(nc) as tc, ExitStack() as es:
        pp = es.enter_context(tc.tile_pool(name="persist", bufs=1))
        dram = es.enter_context(tc.tile_pool(name="dram", bufs=1, space="DRAM"))
        sp = es.enter_context(tc.tile_pool(name="work", bufs=2))
        scr = es.enter_context(tc.tile_pool(name="scr", bufs=1))

        # ---- W1 AllGather into remapped node layout ----
        # (collectives cannot read IO tensors -> stage via internal DRAM)
        w1_in = dram.tile([NBP, NHID], bf16, name="w1_in")
        nc.sync.dma_start(w1_in[:], w1s[:])
        w1_full = dram.tile([NC * NBP, NHID], bf16, addr_space="Shared",
                            name="w1_full")
        nc.gpsimd.collective_compute(
            "AllGather", OP.bypass, replica_groups=rg,
            ins=[w1_in.opt()], outs=[w1_full.opt()])

        # ---- weights / constants from blobb ----
        def ldb(key, rows, cols, name):
            t_ = pp.tile([rows, cols], bf16, name=name, tag=name)
            nc.sync.dma_start(t_[:], blobb[0:rows, voff[key]:voff[key] + cols])
            return t_

        def ldf(key, rows, cols, name, prow=0):
            s_ = scr.tile([rows, cols], bf16, tag="c_" + name)
            nc.sync.dma_start(
                s_[:], blobb[prow:prow + rows, voff[key]:voff[key] + cols])
            t_ = pp.tile([rows, cols], f32, name=name, tag=name)
            nc.scalar.copy(t_[:], s_[:])
            return t_

        W2 = ldb("W2", NHID, NOUT, "W2")
        wihrz = ldb("wihrz", NOUT, 2 * NG, "wihrz")
        whhrz = ldb("whhrz", NG, 2 * NG, "whhrz")
        wihn = ldb("wihn", NOUT, NG, "wihn")
        whhn = ldb("whhn", NG, NG, "whhn")
        npw1 = ldb("npw1", 2 * NG, NG, "npw1")
        npw2 = ldb("npw2", NG, 2, "npw2")
        iota = ldf("iota", P, P, "iota")
        ident = ldf("ident", P, P, "ident")
        a1rep = ldf("a1rep", P, NG, "a1rep")
        a2rep = ldf("a2rep", P, NG, "a2rep")
        b1 = ldf("b1", NHID, 1, "b1")
        brz = ldf("brz", 2 * NG, 1, "brz")
        b2 = ldf("b2", NOUT, 1, "b2")
        brzz = ldf("brzz", NG, 1, "brzz")
        bihn = ldf("bihn", NG, 1, "bihn")
        bhhn = ldf("bhhn", NG, 1, "bhhn")
        npb1 = ldf("npb1", NG, 1, "npb1")
        bng = ldf("bng", NG, 1, "bng")
        bnb = ldf("bnb", NG, 1, "bnb")
        npb2a = ldf("npb2", 1, 1, "npb2a", prow=0)
        npb2b = ldf("npb2", 1, 1, "npb2b", prow=1)

        epsap = pp.tile([NG, 1], f32)
        nc.vector.memset(epsap[:], BN_EPS)
        attnbap = pp.tile([P, 1], f32)
        nc.vector.memset(attnbap[:], attn_b)
        h = pp.tile([NG, NBP], f32)
        nc.vector.memset(h[:], 0.0)
        x2bf = pp.tile([NOUT, NBP], bf16)
        zT = pp.tile([2 * NG, NBP], bf16)
        ystage = pp.tile([P, NB, NHID], bf16)
        nc.vector.memset(ystage[:], 0.0)

        y_in = [dram.tile([NBP, NHID], bf16, name=f"y_in{i}") for i in range(T)]
        y_full = [dram.tile([NC * NBP, NHID], bf16, addr_space="Shared",
                            name=f"y_full{i}") for i in range(T)]
        femb_loc = dram.tile([NBP, NHID], bf16)
        femb_full = dram.tile([NC * NBP, NHID], bf16, addr_space="Shared")
        bn_in = dram.tile([NG, 2], f32)
        bn_out = dram.tile([NG, 2], f32, addr_space="Shared")

        vaS = pp.tile([P, CA], bf16)
        vbS = pp.tile([P, CB], bf16)
        laS = pp.tile([P, CA], f32)
        lbS = pp.tile([P, CB], f32)

        def repl_idx(dst, key, w):
            """Replicate compact [16, w] idx into [128, w] (8 groups)."""
            for g in range(8):
                nc.sync.dma_start(dst[16 * g:16 * g + 16, :w],
                                  blobi[:, ioff[key]:ioff[key] + w])

        def cvt(dst, key, cols):
            """DMA bf16 day data and convert to f32."""
            s_ = scr.tile([P, cols], bf16, tag="cv_" + key[0] if isinstance(
                key, tuple) else "cv_" + key, name="cvt")
            nc.sync.dma_start(s_[:], blobb[:, voff[key]:voff[key] + cols])
            nc.scalar.copy(dst[:], s_[:])

        def onehot(dst, lr_sl, val_sl):
            nt = dst.shape[1]
            nc.vector.tensor_tensor(
                out=dst[:], in0=iota[:, None, :].to_broadcast([P, nt, P]),
                in1=lr_sl[:, :, None].to_broadcast([P, nt, P]),
                op=OP.is_equal)
            if val_sl is not None:
                nc.vector.tensor_tensor(
                    out=dst[:], in0=dst[:],
                    in1=val_sl[:, :, None].to_broadcast([P, nt, P]),
                    op=OP.mult)

        def gather(dst, src_ap, ixS, off16, nidx, elem):
            # single_packet coalesces each engine's descs into one packet
            # (<=64 descs) -> cap each call at 1024 indices
            nt = nidx // P
            SUB = 8
            for s0 in range(0, nt, SUB):
                st = min(SUB, nt - s0)
                nc.gpsimd.dma_gather(dst[:, s0:s0 + st, :], src_ap,
                                     ixS[:, off16 + s0 * 8:off16 + (s0 + st) * 8],
                                     st * P, st * P, elem)

        def spmm(ps, iaT, ibT, ta, tb, srcA, srcB, elem, out_cb, tag_pb,
                 pdim, laT, lbT, vaT, vbT):
            """Band-split gather + one-hot matmul scatter over all blocks."""
            for ch in range(NCHUNK):
                ntA, ntB = CH * ta, CH * tb
                gA = sp.tile([P, ntA, elem], bf16, tag="gA")
                gather(gA, srcA, iaT, ch * ntA * 8, ntA * P, elem)
                gB = sp.tile([P, ntB, elem], bf16, tag="gB")
                gather(gB, srcB, ibT, ch * ntB * 8, ntB * P, elem)
                ohA = sp.tile([P, ntA, P], bf16, tag="ohA")
                onehot(ohA, laT[:, ch * ntA:(ch + 1) * ntA],
                       vaT[:, ch * ntA:(ch + 1) * ntA] if vaT is not None
                       else None)
                ohB = sp.tile([P, ntB, P], bf16, tag="ohB")
                onehot(ohB, lbT[:, ch * ntB:(ch + 1) * ntB],
                       vbT[:, ch * ntB:(ch + 1) * ntB] if vbT is not None
                       else None)
                for j in range(CH):
                    b = ch * CH + j
                    pb = ps.tile([pdim, P], f32, tag=tag_pb, space="PSUM")
                    for a in range(ta):
                        nc.tensor.matmul(
                            pb[:], lhsT=gA[:, j * ta + a, :pdim],
                            rhs=ohA[:, j * ta + a, :],
                            start=(a == 0), stop=False)
                    for bb in range(tb):
                        nc.tensor.matmul(
                            pb[:], lhsT=gB[:, j * tb + bb, :pdim],
                            rhs=ohB[:, j * tb + bb, :],
                            start=False, stop=(bb == tb - 1))
                    out_cb(b, pb)

        WAm, WBm = max(WA, WA7), max(WB, WB7)

        # ================= time steps =================
        for t in range(T):
            iaS = scr.tile([P, WAm], i16, tag="iaS")
            repl_idx(iaS, ("ia", t), WA)
            ibS = scr.tile([P, WBm], i16, tag="ibS")
            repl_idx(ibS, ("ib", t), WB)
            nc.sync.dma_start(vaS[:], blobb[:, voff[("va", t)]:
                                            voff[("va", t)] + CA])
            nc.sync.dma_start(vbS[:], blobb[:, voff[("vb", t)]:
                                            voff[("vb", t)] + CB])
            cvt(laS, ("la", t), CA)
            cvt(lbS, ("lb", t), CB)

            # ---- spmm1 + fused y = relu(.)@W2, transposed staging ----
            with tc.tile_pool(name=f"ps1_{t}", bufs=2, space="PSUM") as ps:
                def close1(b, pb, ps=ps):
                    x1b = sp.tile([NHID, P], bf16, tag="x1b")
                    nc.scalar.activation(x1b[:], pb[:], AF.Relu, bias=b1[:])
                    py = ps.tile([NOUT, P], f32, tag="py", space="PSUM")
                    nc.tensor.matmul(py[:], lhsT=W2[:], rhs=x1b[:],
                                     start=True, stop=True)
                    ysb = sp.tile([NOUT, P], f32, tag="ysb")
                    nc.scalar.copy(ysb[:], py[:])
                    pyt = ps.tile([P, NOUT], f32, tag="pyt", space="PSUM")
                    nc.tensor.transpose(pyt[:], ysb[:], ident[:NOUT, :NOUT])
                    nc.scalar.copy(ystage[:, b, :NOUT], pyt[:])
                spmm(ps, iaS, ibS, TA, TB, w1_full[:, :], w1_full[m.PIV2:, :],
                     NHID, close1, "pb", NHID, laS, lbS, vaS, vbS)

            nc.sync.dma_start(
                y_in[t][:].rearrange("(b p) d -> p b d", p=P), ystage[:])
            nc.gpsimd.collective_compute(
                "AllGather", OP.bypass, replica_groups=rg,
                ins=[y_in[t].opt()], outs=[y_full[t].opt()])

            # ---- spmm2 ----
            with tc.tile_pool(name=f"ps2_{t}", bufs=2, space="PSUM") as ps:
                yf = y_full[t]
                def close2(b, pb):
                    nc.scalar.activation(
                        x2bf[:, b * P:(b + 1) * P], pb[:], AF.Identity,
                        bias=b2[:])
                spmm(ps, iaS, ibS, TA, TB, yf[:, :], yf[m.PIV2:, :],
                     NHID, close2, "pb2", NOUT, laS, lbS, vaS, vbS)

            # ---- GRU ----
            with tc.tile_pool(name=f"psg_{t}", bufs=2, space="PSUM") as ps:
                CL = 512
                for s in range(0, NBP, CL):
                    L = min(CL, NBP - s)
                    hbfc = scr.tile([NG, CL], bf16, tag="hbfc")
                    nc.scalar.copy(hbfc[:, :L], h[:, s:s + L])
                    prz = ps.tile([2 * NG, CL], f32, tag="prz", space="PSUM")
                    nc.tensor.matmul(prz[:, :L], lhsT=wihrz[:],
                                     rhs=x2bf[:, s:s + L], start=True,
                                     stop=False)
                    nc.tensor.matmul(prz[:, :L], lhsT=whhrz[:],
                                     rhs=hbfc[:, :L], start=False,
                                     stop=True)
                    rzr = sp.tile([NG, CL], f32, tag="rzr")
                    nc.scalar.activation(rzr[:, :L], prz[:NG, :L], AF.Sigmoid,
                                         bias=brz[:NG])
                    rzz = sp.tile([NG, CL], f32, tag="rzz")
                    nc.scalar.activation(rzz[:, :L], prz[NG:, :L], AF.Sigmoid,
                                         bias=brzz[:])
                    pn = ps.tile([NG, CL], f32, tag="pn", space="PSUM")
                    nc.tensor.matmul(pn[:, :L], lhsT=wihn[:],
                                     rhs=x2bf[:, s:s + L], start=True,
                                     stop=True)
                    phn = ps.tile([NG, CL], f32, tag="phn", space="PSUM")
                    nc.tensor.matmul(phn[:, :L], lhsT=whhn[:],
                                     rhs=hbfc[:, :L], start=True,
                                     stop=True)
                    ghn = scr.tile([NG, CL], f32, tag="ghn")
                    nc.scalar.activation(ghn[:, :L], phn[:, :L], AF.Identity,
                                         bias=bhhn[:])
                    t1 = scr.tile([NG, CL], f32, tag="t1")
                    nc.vector.tensor_tensor(out=t1[:, :L], in0=rzr[:, :L],
                                            in1=ghn[:, :L], op=OP.mult)
                    t2 = scr.tile([NG, CL], f32, tag="t2")
                    nc.vector.tensor_tensor(out=t2[:, :L], in0=t1[:, :L],
                                            in1=pn[:, :L], op=OP.add)
                    nsb = scr.tile([NG, CL], f32, tag="nsb")
                    nc.scalar.activation(nsb[:, :L], t2[:, :L], AF.Tanh,
                                         bias=bihn[:])
                    dd = scr.tile([NG, CL], f32, tag="t2", name="dd")
                    nc.vector.tensor_tensor(out=dd[:, :L], in0=h[:, s:s + L],
                                            in1=nsb[:, :L], op=OP.subtract)
                    zd = scr.tile([NG, CL], f32, tag="t1", name="zd")
                    nc.vector.tensor_tensor(out=zd[:, :L], in0=rzz[:, :L],
                                            in1=dd[:, :L], op=OP.mult)
                    nc.vector.tensor_tensor(out=h[:, s:s + L], in0=nsb[:, :L],
                                            in1=zd[:, :L], op=OP.add)

        # ================= BatchNorm =================
        hsum = pp.tile([NG, 1], f32)
        nc.vector.tensor_reduce(out=hsum[:], in_=h[:, :NL],
                                axis=mybir.AxisListType.X, op=OP.add)
        hsq = pp.tile([NG, 1], f32)
        nc.scalar.activation(x2bf[:, :NL], h[:, :NL], AF.Square,
                             accum_out=hsq[:])
        bnsb = pp.tile([NG, 2], f32)
        nc.vector.tensor_copy(bnsb[:, 0:1], hsum[:])
        nc.vector.tensor_copy(bnsb[:, 1:2], hsq[:])
        nc.sync.dma_start(bn_in[:], bnsb[:])
        nc.gpsimd.collective_compute(
            "AllReduce", OP.add, replica_groups=rg,
            ins=[bn_in.opt()], outs=[bn_out.opt()])
        bnrs = pp.tile([NG, 2], f32)
        nc.sync.dma_start(bnrs[:], bn_out[:])
        mean = pp.tile([NG, 1], f32)
        nc.scalar.mul(mean[:], bnrs[:, 0:1], 1.0 / N)
        ex2 = pp.tile([NG, 1], f32)
        nc.scalar.mul(ex2[:], bnrs[:, 1:2], 1.0 / N)
        msq = pp.tile([NG, 1], f32)
        nc.scalar.activation(msq[:], mean[:], AF.Square)
        var = pp.tile([NG, 1], f32)
        nc.vector.tensor_tensor(out=var[:], in0=ex2[:], in1=msq[:],
                                op=OP.subtract)
        sd = pp.tile([NG, 1], f32)
        nc.scalar.activation(sd[:], var[:], AF.Sqrt, bias=epsap[:])
        inv = pp.tile([NG, 1], f32)
        nc.vector.reciprocal(inv[:], sd[:])
        scale = pp.tile([NG, 1], f32)
        nc.vector.tensor_tensor(out=scale[:], in0=bng[:], in1=inv[:],
                                op=OP.mult)
        mscale = pp.tile([NG, 1], f32)
        nc.vector.tensor_tensor(out=mscale[:], in0=mean[:], in1=scale[:],
                                op=OP.mult)
        shift = pp.tile([NG, 1], f32)
        nc.vector.tensor_tensor(out=shift[:], in0=bnb[:], in1=mscale[:],
                                op=OP.subtract)
        nc.scalar.activation(h[:], h[:], AF.Identity, bias=shift[:],
                             scale=scale[:])
        nc.scalar.copy(zT[:NG, :], h[:])
        with tc.tile_pool(name="psT", bufs=2, space="PSUM") as psT:
            for b in range(NB):
                pyt = psT.tile([P, NG], f32, tag="pyt2", space="PSUM")
                nc.tensor.transpose(pyt[:], h[:, b * P:(b + 1) * P],
                                    ident[:NG, :NG])
                nc.scalar.copy(ystage[:, b, :NOUT], pyt[:])
        nc.sync.dma_start(
            femb_loc[:].rearrange("(b p) d -> p b d", p=P), ystage[:])
        nc.gpsimd.collective_compute(
            "AllGather", OP.bypass, replica_groups=rg,
            ins=[femb_loc.opt()], outs=[femb_full.opt()])

        # ================= attention readout =================
        v7aS = pp.tile([P, CA7], f32)
        v7bS = pp.tile([P, CB7], f32)
        l7aS = pp.tile([P, CA7], f32)
        l7bS = pp.tile([P, CB7], f32)
        cvt(v7aS, "v7a", CA7)
        cvt(v7bS, "v7b", CB7)
        cvt(l7aS, "l7a", CA7)
        cvt(l7bS, "l7b", CB7)
        i7aS = scr.tile([P, WAm], i16, tag="iaS", name="i7aS")
        repl_idx(i7aS, "i7a", WA7)
        i7bS = scr.tile([P, WBm], i16, tag="ibS", name="i7bS")
        repl_idx(i7bS, "i7b", WB7)

        def repl_idx_chunk(key, off16, n16, tag):
            ix = sp.tile([P, n16], i16, tag=tag)
            for g in range(8):
                nc.sync.dma_start(
                    ix[16 * g:16 * g + 16, :],
                    blobi[:, ioff[key] + off16:ioff[key] + off16 + n16])
            return ix

        with tc.tile_pool(name="psA", bufs=2, space="PSUM") as ps:
            for ch in range(NCHUNK):
                tiles = {}
                for sfx, nt, tt, icol, irkey, vS, lS, src in (
                        ("A", CH * TA7, TA7, i7aS, "i7ra", v7aS, l7aS,
                         femb_full[:, :]),
                        ("B", CH * TB7, TB7, i7bS, "i7rb", v7bS, l7bS,
                         femb_full[m.PIV2:, :])):
                    gC = sp.tile([P, nt, NHID], bf16, tag="g" + sfx)
                    gather(gC, src, icol, ch * nt * 8, nt * P, NHID)
                    irx = repl_idx_chunk(irkey, ch * nt * 8, nt * 8,
                                         "ir" + sfx)
                    gR = scr.tile([P, nt, NHID], bf16, tag="gR" + sfx)
                    gather(gR, femb_loc[:, :], irx, 0, nt * P, NHID)
                    oh = sp.tile([P, nt, P], bf16, tag="oh" + sfx)
                    onehot(oh, lS[:, ch * nt:(ch + 1) * nt], None)
                    mm = scr.tile([P, nt, NOUT], bf16, tag="mscr")
                    nc.vector.tensor_tensor(
                        out=mm[:], in0=gR[:, :, :NOUT],
                        in1=a1rep[:, None, :].to_broadcast([P, nt, NOUT]),
                        op=OP.mult)
                    s1 = sp.tile([P, nt], f32, tag="s1")
                    nc.vector.tensor_reduce(out=s1[:], in_=mm[:],
                                            axis=mybir.AxisListType.X,
                                            op=OP.add)
                    nc.vector.tensor_tensor(
                        out=mm[:], in0=gC[:, :, :NOUT],
                        in1=a2rep[:, None, :].to_broadcast([P, nt, NOUT]),
                        op=OP.mult)
                    s2 = sp.tile([P, nt], f32, tag="s2")
                    nc.vector.tensor_reduce(out=s2[:], in_=mm[:],
                                            axis=mybir.AxisListType.X,
                                            op=OP.add)
                    nc.vector.tensor_tensor(out=s1[:], in0=s1[:], in1=s2[:],
                                            op=OP.add)
                    wv = sp.tile([P, nt], f32, tag="wv" + sfx)
                    nc.scalar.activation(wv[:], s1[:], AF.Sigmoid,
                                         bias=attnbap[:])
                    nc.vector.tensor_tensor(
                        out=wv[:], in0=wv[:],
                        in1=vS[:, ch * nt:(ch + 1) * nt], op=OP.mult)
                    for ti in range(nt):
                        nc.scalar.activation(gC[:, ti, NOUT:2 * NOUT],
                                             gC[:, ti, :NOUT],
                                             AF.Copy, scale=wv[:, ti:ti + 1])
                    tiles[sfx] = (gC, oh, tt)
                for j in range(CH):
                    b = ch * CH + j
                    pnb = ps.tile([NOUT, P], f32, tag="pnb", space="PSUM")
                    cbf, oh, tt = tiles["A"]
                    for a in range(tt):
                        nc.tensor.matmul(
                            pnb[:], lhsT=cbf[:, j * tt + a, NOUT:2 * NOUT],
                            rhs=oh[:, j * tt + a, :],
                            start=(a == 0), stop=False)
                    cbf, oh, tt = tiles["B"]
                    for bb in range(tt):
                        nc.tensor.matmul(
                            pnb[:], lhsT=cbf[:, j * tt + bb, NOUT:2 * NOUT],
                            rhs=oh[:, j * tt + bb, :],
                            start=False, stop=(bb == tt - 1))
                    nc.scalar.copy(zT[NG:, b * P:(b + 1) * P], pnb[:])

        # ================= final MLP + log_softmax =================
        with tc.tile_pool(name="psF", bufs=2, space="PSUM") as ps:
            CL = 128
            for s in range(0, NBP, CL):
                L = min(CL, NBP - s)
                ph1 = ps.tile([NG, CL], f32, tag="ph1", space="PSUM")
                nc.tensor.matmul(ph1[:, :L], lhsT=npw1[:], rhs=zT[:, s:s + L],
                                 start=True, stop=True)
                h1b = sp.tile([NG, CL], bf16, tag="h1b")
                nc.scalar.activation(h1b[:, :L], ph1[:, :L], AF.Relu,
                                     bias=npb1[:])
                ps2a = ps.tile([1, CL], f32, tag="ps2a", space="PSUM")
                nc.tensor.matmul(ps2a[:, :L], lhsT=npw2[:, 0:1],
                                 rhs=h1b[:, :L], start=True, stop=True)
                s0 = scr.tile([1, CL], f32, tag="lsm_s0")
                nc.scalar.activation(s0[:, :L], ps2a[:, :L],
                                     AF.Identity, bias=npb2a[:])
                ps2b = ps.tile([1, CL], f32, tag="ps2b", space="PSUM")
                nc.tensor.matmul(ps2b[:, :L], lhsT=npw2[:, 1:2],
                                 rhs=h1b[:, :L], start=True, stop=True)
                s1c = scr.tile([1, CL], f32, tag="lsm_s1")
                nc.scalar.activation(s1c[:, :L], ps2b[:, :L],
                                     AF.Identity, bias=npb2b[:])
                if s >= NL:
                    continue
                Lv = min(L, NL - s)
                mx = scr.tile([1, CL], f32, tag="lsm_mx")
                nc.vector.tensor_tensor(out=mx[:, :L], in0=s0[:, :L],
                                        in1=s1c[:, :L], op=OP.max)
                sh0 = scr.tile([1, CL], f32, tag="lsm_sh0")
                nc.vector.tensor_tensor(out=sh0[:, :L], in0=s0[:, :L],
                                        in1=mx[:, :L], op=OP.subtract)
                sh1 = scr.tile([1, CL], f32, tag="lsm_sh1")
                nc.vector.tensor_tensor(out=sh1[:, :L], in0=s1c[:, :L],
                                        in1=mx[:, :L], op=OP.subtract)
                e0 = scr.tile([1, CL], f32, tag="lsm_s0")
                nc.scalar.activation(e0[:, :L], sh0[:, :L], AF.Exp)
                e1 = scr.tile([1, CL], f32, tag="lsm_s1")
                nc.scalar.activation(e1[:, :L], sh1[:, :L], AF.Exp)
                se = scr.tile([1, CL], f32, tag="lsm_mx")
                nc.vector.tensor_tensor(out=se[:, :L], in0=e0[:, :L],
                                        in1=e1[:, :L], op=OP.add)
                lg = scr.tile([1, CL], f32, tag="lsm_s0")
                nc.scalar.activation(lg[:, :L], se[:, :L], AF.Ln)
                p0 = scr.tile([1, CL], f32, tag="lsm_s1")
                nc.vector.tensor_tensor(out=p0[:, :L], in0=sh0[:, :L],
                                        in1=lg[:, :L], op=OP.subtract)
                p1 = scr.tile([1, CL], f32, tag="lsm_mx")
                nc.vector.tensor_tensor(out=p1[:, :L], in0=sh1[:, :L],
                                        in1=lg[:, :L], op=OP.subtract)
                nc.sync.dma_start(pred_out[0:1, s:s + Lv], p0[:, :Lv])
                nc.sync.dma_start(pred_out[1:2, s:s + Lv], p1[:, :Lv])

    nc.compile()
    return nc


# ----------------------------------------------------------------------------
# entry point
# ----------------------------------------------------------------------------

def make_in_maps(inputs, m, percore):
    W1 = np.asarray(inputs["W1"], np.float32)
    W2 = np.asarray(inputs["W2"], np.float32)
    NG = W2.shape[1]
    NHID = W1.shape[1]
    w_ih = np.asarray(inputs["w_ih"], np.float32)
    w_hh = np.asarray(inputs["w_hh"], np.float32)
    b_ih = np.asarray(inputs["b_ih"], np.float32)
    b_hh = np.asarray(inputs["b_hh"], np.float32)
    attn_w = np.asarray(inputs["attn_w"], np.float32)
    voff, WBF = val_layout(m)

    wsec = np.zeros((P, WBF), BF)

    def put(key, a):
        a = np.asarray(a, np.float32)
        wsec[:a.shape[0], voff[key]:voff[key] + a.shape[1]] = a.astype(BF)

    put("W2", W2)
    put("wihrz", np.ascontiguousarray(w_ih[:2 * NG].T))
    put("whhrz", np.ascontiguousarray(w_hh[:2 * NG].T))
    put("wihn", np.ascontiguousarray(w_ih[2 * NG:].T))
    put("whhn", np.ascontiguousarray(w_hh[2 * NG:].T))
    put("npw1", np.asarray(inputs["np_w1"], np.float32))
    put("npw2", np.asarray(inputs["np_w2"], np.float32))
    put("iota", np.broadcast_to(np.arange(P, dtype=np.float32), (P, P)))
    put("ident", np.eye(P, dtype=np.float32))
    put("a1rep", np.broadcast_to(attn_w[:NG, 0], (P, NG)))
    put("a2rep", np.broadcast_to(attn_w[NG:, 0], (P, NG)))
    put("b1", np.asarray(inputs["b1"], np.float32).reshape(-1, 1))
    put("brz", (b_ih[:2 * NG] + b_hh[:2 * NG]).reshape(-1, 1))
    put("b2", np.asarray(inputs["b2"], np.float32).reshape(-1, 1))
    put("brzz", (b_ih[NG:2 * NG] + b_hh[NG:2 * NG]).reshape(-1, 1))
    put("bihn", b_ih[2 * NG:].reshape(-1, 1))
    put("bhhn", b_hh[2 * NG:].reshape(-1, 1))
    put("npb1", np.asarray(inputs["np_b1"], np.float32).reshape(-1, 1))
    put("bng", np.asarray(inputs["bn_gamma"], np.float32).reshape(-1, 1))
    put("bnb", np.asarray(inputs["bn_beta"], np.float32).reshape(-1, 1))
    put("npb2", np.asarray(inputs["np_b2"], np.float32).reshape(-1, 1))

    wstart = voff["W2"]                    # weights region is contiguous

    in_maps = []
    for k in range(m.NC):
        blobi, blobb = percore[k]
        blobb = blobb.copy()
        blobb[:, wstart:] = wsec[:, wstart:]
        w1pad = np.zeros((m.NBP, NHID), BF)
        w1pad[:m.NL] = W1[k * m.NL:(k + 1) * m.NL].astype(BF)
        in_maps.append({"blobi": blobi, "blobb": blobb, "w1s": w1pad})
    return in_maps


_CACHE = {}
LAST_RESULTS = None


def kernel(**inputs):
    n_cores = 8
    m, percore = preprocess(inputs, n_cores)
    in_maps = make_in_maps(inputs, m, percore)
    key = (m.N, m.T, m.TA, m.TB, m.TA7, m.TB7)
    if key not in _CACHE:
        NHID = int(np.asarray(inputs["W1"]).shape[1])
        NOUT = int(np.asarray(inputs["W2"]).shape[1])
        attn_b = float(np.asarray(inputs["attn_b"]).reshape(-1)[0])
        _CACHE[key] = build_program(m, NHID, NOUT, attn_b)
    nc = _CACHE[key]
    res = run_bass_kernel_spmd(nc, in_maps, list(range(n_cores)))
    global LAST_RESULTS
    LAST_RESULTS = res
    pred = np.concatenate(
        [res.results[k]["pred"].T for k in range(n_cores)], axis=0)
    return np.ascontiguousarray(pred.astype(np.float32))


if __name__ == "__main__":
    import reference as R
    inputs = {k: np.asarray(v) for k, v in R.setup_inputs().items()}
    out = kernel(**inputs)
    print(out.shape, out.dtype, out[:2])
